# revision 16
# baseline (speedup 1.0000x reference)
"""AVWGCN (adaptive vertex-wise graph conv) Trainium2 kernel.

Math (reference):
  e  = LayerNorm(node_embeddings)                      [N, D]
  S  = softmax(elu(e @ e.T), axis=0)                   [N, N]
  supports = [I, S, 2*S@S - I]
  W  = einsum('nd,dkio->nkio', e, weights_pool)        [N, K, Din, Dout]
  b  = e @ bias_pool                                   [N, Dout]
  x_g = einsum('knm,bmc->bnkc', supports, x)           [B, N, K, Din]
  out = einsum('bnki,nkio->bno', x_g, W) + b           [B, N, Dout]

Never materialize S@S: x_g1 = S @ x; x_g2 = 2*S@x_g1 - x.

Sharding: node rows of S across 8 cores (512 each). Column-softmax
denominators via 16KB AllReduce; x_g1 via bf16 AllGather.

exp(elu(g)) computed Exp-only (no act-table swaps):
  t = exp(g); r = max(t, exp(min(t,1) - 1)).

x_g2 is computed directly in transposed layout (lhsT = gathered-xg1
column group, rhs = S^T tile), folding 2*ps - x^T against the x^T rows
already sitting in xgT.  Per-node conv: lhsT = W^T node block (o-major),
rhs = x_g^T node column, output [o, (n, b)] stored as [DOUT, NBLK, B]
in DRAM; the host transposes to [B, NBLK, DOUT] when unsharding.
"""

import numpy as np

N = 4096
D = 16          # embed
DIN = 32
DOUT = 64
CHEB_K = 3
B = 16
NCORES = 8
NBLK = N // NCORES          # 512 nodes per core
BC = B * DIN                # 512
MT = N // 128               # 32 m tiles
KI = CHEB_K * DIN           # 96
LN_EPS = 1e-12

_CACHE = {}


def _build_program():
    import concourse.bass as bass
    import concourse.bacc as bacc
    import concourse.mybir as mybir
    import concourse.tile as tile
    from contextlib import ExitStack

    f32 = mybir.dt.float32
    f32r = mybir.dt.float32r
    bf16 = mybir.dt.bfloat16
    AF = mybir.ActivationFunctionType
    ALU = mybir.AluOpType
    AX = mybir.AxisListType

    nc = bacc.Bacc(
        "TRN2", target_bir_lowering=False, debug=False, num_devices=NCORES
    )

    # -------- DRAM inputs (host-prepped layouts) --------
    x_t_d = nc.dram_tensor("x_t", [N, BC], bf16, kind="ExternalInput")
    # x^T of own block: [c, (n, b)]
    xTb_d = nc.dram_tensor("xTb", [DIN, B * NBLK], bf16, kind="ExternalInput")
    ne_re_d = nc.dram_tensor("ne_re", [128, MT * D], f32, kind="ExternalInput")
    neb_re_d = nc.dram_tensor("neb_re", [128, (NBLK // 128) * D], f32, kind="ExternalInput")
    wpb_d = nc.dram_tensor("wpb", [D, DOUT * (KI + 1)], bf16, kind="ExternalInput")
    gam_d = nc.dram_tensor("gam", [D], f32, kind="ExternalInput")
    bet_d = nc.dram_tensor("bet", [D], f32, kind="ExternalInput")
    ident_d = nc.dram_tensor("ident", [128, 128], f32, kind="ExternalInput")
    identb_d = nc.dram_tensor("identb", [128, 128], bf16, kind="ExternalInput")
    out_d = nc.dram_tensor("out_blk", [B, NBLK, DOUT], f32, kind="ExternalOutput")

    # internal DRAM for collectives
    cs_in_a = nc.dram_tensor("cs_in_a", [128, MT // 2], f32)
    cs_out_a = nc.dram_tensor("cs_out_a", [128, MT // 2], f32, addr_space="Shared")
    cs_in_b = nc.dram_tensor("cs_in_b", [128, MT // 2], f32)
    cs_out_b = nc.dram_tensor("cs_out_b", [128, MT // 2], f32, addr_space="Shared")
    ag_in = nc.dram_tensor("ag_in", [NBLK, BC], bf16)
    ag_out = nc.dram_tensor("ag_out", [N, BC], bf16, addr_space="Shared")

    rg = [list(range(NCORES))]

    with tile.TileContext(nc) as tc, ExitStack() as ctx:
        persist = ctx.enter_context(tc.tile_pool(name="persist", bufs=1))
        work = ctx.enter_context(tc.tile_pool(name="work", bufs=3))
        psA = ctx.enter_context(tc.tile_pool(name="psA", bufs=3, space="PSUM"))
        psB = ctx.enter_context(tc.tile_pool(name="psB", bufs=2, space="PSUM"))
        psBb = ctx.enter_context(tc.tile_pool(name="psBb", bufs=1, space="PSUM"))
        psC = ctx.enter_context(tc.tile_pool(name="psC", bufs=2, space="PSUM"))
        outp = ctx.enter_context(tc.tile_pool(name="outp", bufs=2))

        # ---------------- persistent loads ----------------
        ident = persist.tile([128, 128], f32, tag="ident")
        nc.sync.dma_start(ident[:], ident_d[:])
        identb = persist.tile([128, 128], bf16, tag="identb")
        nc.sync.dma_start(identb[:], identb_d[:])
        eps_sb = persist.tile([128, 1], f32, tag="eps")
        nc.vector.memset(eps_sb[:], LN_EPS)
        neg1_sb = persist.tile([128, 1], f32, tag="neg1")
        nc.vector.memset(neg1_sb[:], -1.0)
        gam_sb = persist.tile([128, D], f32, tag="gam")
        nc.sync.dma_start(gam_sb[:], gam_d[:].unsqueeze(0).broadcast_to([128, D]))
        bet_sb = persist.tile([128, D], f32, tag="bet")
        nc.sync.dma_start(bet_sb[:], bet_d[:].unsqueeze(0).broadcast_to([128, D]))
        wpb_sb = persist.tile([D, DOUT * (KI + 1)], bf16, tag="wpb")
        nc.sync.dma_start(wpb_sb[:], wpb_d[:])
        ne_sb = persist.tile([128, MT, D], f32, tag="ne_sb")
        nc.sync.dma_start(ne_sb[:], ne_re_d[:].rearrange("p (t d) -> p t d", d=D))
        neb_sb = persist.tile([128, NBLK // 128, D], f32, tag="neb_sb")
        nc.sync.dma_start(neb_sb[:], neb_re_d[:].rearrange("p (t d) -> p t d", d=D))

        # x_g^T tile: rows (k,i) + ones row; cols = 16*node + b
        xgT = persist.tile([KI + 1, B * NBLK], bf16, tag="xgT")
        nc.scalar.dma_start(xgT[0:DIN, :], xTb_d[:])
        nc.vector.memset(xgT[KI : KI + 1, :], 1.0)

        # x tiles [m-part, (b c)] bf16; later overwritten with gathered xg1
        xt_re = x_t_d.rearrange("(t p) f -> t p f", p=128)
        x_sb = []
        for t in range(MT):
            xt = persist.tile([128, BC], bf16, tag=f"xs{t}")
            nc.sync.dma_start(xt[:], xt_re[t])
            x_sb.append(xt)

        # col = n_hi*256 + b*16 + n_lo  (32B-contiguous n_lo runs)
        xgT_k0 = xgT[0:DIN, :].rearrange("p (h b l) -> p h b l", h=32, b=B)
        xgT_k1 = xgT[DIN : 2 * DIN, :].rearrange("p (h b l) -> p h b l", h=32, b=B)
        xgT_k2 = xgT[2 * DIN : 3 * DIN, :].rearrange("p (h b l) -> p h b l", h=32, b=B)

        # W^T: rows (k,i)+bias; cols = n_hi*1024 + o*16 + n_lo
        wt = persist.tile([KI + 1, DOUT * NBLK], bf16, tag="wt")
        wt_v = wt[:].rearrange("p (h o l) -> p h o l", h=32, o=DOUT)

        ebT = persist.tile([D, NBLK], f32r, tag="ebT")
        ebT_bf = persist.tile([D, NBLK], bf16, tag="ebT_bf")
        cs_part = persist.tile([128, MT], f32, tag="cs_part")
        rcol = persist.tile([128, MT], f32, tag="rcol")
        etn = [persist.tile([128, NBLK], bf16, tag=f"etn{t}", name=f"etn{t}") for t in range(MT)]
        xg1_bf = [persist.tile([128, BC], bf16, tag=f"xg1_{j}", name=f"xg1_{j}") for j in range(4)]

        # ---------------- batched layernorm ----------------
        def layernorm_batch(src, dst, nt):
            # src/dst: [128, nt, D]
            mu = work.tile([128, nt], f32, tag="ln_mu")
            nc.vector.tensor_reduce(mu[:], src, axis=AX.X, op=ALU.add)
            nc.vector.tensor_scalar_mul(mu[:], mu[:], 1.0 / D)
            muB = mu[:].unsqueeze(-1).broadcast_to([128, nt, D])
            cen = work.tile([128, nt, D], f32, tag="ln_cen")
            nc.vector.tensor_tensor(cen[:], src, muB, ALU.subtract)
            sq = work.tile([128, nt, D], f32, tag="ln_sq")
            nc.vector.tensor_tensor(sq[:], cen[:], cen[:], ALU.mult)
            ssq = work.tile([128, nt], f32, tag="ln_ssq")
            nc.vector.tensor_reduce(ssq[:], sq[:], axis=AX.X, op=ALU.add)
            sd = work.tile([128, nt], f32, tag="ln_sd")
            nc.scalar.activation(sd[:], ssq[:], AF.Sqrt, bias=eps_sb[:], scale=1.0 / D)
            rstd = work.tile([128, nt], f32, tag="ln_rstd")
            nc.vector.reciprocal(rstd[:], sd[:])
            rstdB = rstd[:].unsqueeze(-1).broadcast_to([128, nt, D])
            e1 = work.tile([128, nt, D], f32, tag="ln_e1")
            nc.vector.tensor_tensor(e1[:], cen[:], rstdB, ALU.mult)
            gamB = gam_sb[:].unsqueeze(1).broadcast_to([128, nt, D])
            betB = bet_sb[:].unsqueeze(1).broadcast_to([128, nt, D])
            nc.vector.tensor_tensor(e1[:], e1[:], gamB, ALU.mult)
            nc.vector.tensor_tensor(dst, e1[:], betB, ALU.add)

        # ============ PHASE 1: layernorms ============
        with tc.tile_pool(name="ph1", bufs=1) as ph1:
            e_blk = ph1.tile([128, NBLK // 128, D], f32, tag="e_blk")
            layernorm_batch(neb_sb[:], e_blk[:], NBLK // 128)
            e_full = ph1.tile([128, MT, D], f32, tag="e_full")
            layernorm_batch(ne_sb[:], e_full[:], MT)

            # transpose own-block e -> ebT [D, NBLK]
            for t in range(NBLK // 128):
                pt = psB.tile([128, 128], f32, tag="tr")
                nc.tensor.transpose(pt[0:D, 0:128], e_blk[:, t, :], ident[:])
                nc.vector.tensor_copy(ebT[:, t * 128 : (t + 1) * 128], pt[0:D, 0:128])
            nc.vector.tensor_copy(ebT_bf[:], ebT[:].bitcast(f32))

            # ============ PHASE 2: scores + exp(elu) (Exp only) ============
            for t in range(MT):
                pt = psB.tile([128, 128], f32, tag="tr")
                nc.tensor.transpose(pt[0:D, 0:128], e_full[:, t, :], ident[:])
                eTt = work.tile([D, 128], f32r, tag="eTt", bufs=3)
                nc.vector.tensor_copy(eTt[:], pt[0:D, 0:128])
                gps = psA.tile([128, NBLK], f32, tag="big")
                nc.tensor.matmul(gps[:], eTt[:], ebT[:], start=True, stop=True)
                t_e = work.tile([128, NBLK], bf16, tag="elu_t", bufs=3)
                nc.scalar.activation(t_e[:], gps[:], AF.Exp)
                mn = work.tile([128, NBLK], bf16, tag="elu_m", bufs=3)
                nc.vector.tensor_scalar_min(mn[:], t_e[:], 1.0)
                v = work.tile([128, NBLK], bf16, tag="elu_v", bufs=3)
                nc.scalar.activation(v[:], mn[:], AF.Exp, bias=neg1_sb[:])
                nc.vector.scalar_tensor_tensor(
                    etn[t][:], t_e[:], 1.0, v[:], ALU.mult, ALU.max,
                    accum_out=cs_part[:, t : t + 1],
                )
                if t == MT // 2 - 1:
                    # first-half colsum AllReduce, hidden under scores tail
                    nc.sync.dma_start(cs_in_a[:], cs_part[:, 0 : MT // 2])
                    nc.gpsimd.collective_compute(
                        "AllReduce", mybir.AluOpType.add, replica_groups=rg,
                        ins=[cs_in_a[:]], outs=[cs_out_a[:]],
                    )

            nc.sync.dma_start(cs_in_b[:], cs_part[:, MT // 2 : MT])
            nc.gpsimd.collective_compute(
                "AllReduce", mybir.AluOpType.add, replica_groups=rg,
                ins=[cs_in_b[:]], outs=[cs_out_b[:]],
            )

            # ---- fill the AllReduce bubble: W^T formation (first half) ----
            wpb_v = wpb_sb[:].rearrange("p (o q) -> p o q", q=KI + 1)

            def wt_form(o):
                wps = psA.tile([128, NBLK], f32, tag="big")
                nc.tensor.matmul(
                    wps[0 : KI + 1, :], wpb_v[:, o, :], ebT_bf[:],
                    start=True, stop=True,
                )
                dst = wt_v[:, :, o, :]
                if o % 2 == 0:
                    nc.vector.tensor_copy(dst, wps[0 : KI + 1, :])
                else:
                    nc.scalar.activation(dst, wps[0 : KI + 1, :], AF.Copy)

            for o in range(DOUT // 2):
                wt_form(o)

            # rcol = 1/colsum (blocks on AllReduce)
            nc.sync.dma_start(rcol[:, 0 : MT // 2], cs_out_a[:])
            nc.sync.dma_start(rcol[:, MT // 2 : MT], cs_out_b[:])
            nc.vector.reciprocal(rcol[:], rcol[:])
            for t in range(MT):
                nc.vector.tensor_scalar_mul(etn[t][:], etn[t][:], rcol[:, t : t + 1])

            # ============ PHASE 3: x_g1 = S_blk @ x ============
            for j in range(NBLK // 128):
                ps = psA.tile([128, BC], f32, tag="big")
                for t in range(MT):
                    nc.tensor.matmul(
                        ps[:],
                        etn[t][:, j * 128 : (j + 1) * 128],
                        x_sb[t][:],
                        start=(t == 0),
                        stop=(t == MT - 1),
                    )
                nc.vector.tensor_copy(xg1_bf[j][:], ps[:])
                nc.sync.dma_start(ag_in[j * 128 : (j + 1) * 128, :], xg1_bf[j][:])

            nc.gpsimd.collective_compute(
                "AllGather", mybir.AluOpType.bypass, replica_groups=rg,
                ins=[ag_in[:]], outs=[ag_out[:]],
            )

            # ---- fill the AllGather bubble: rest of W^T + k=1 transposes ----
            for o in range(DOUT // 2, DOUT):
                wt_form(o)

            for j in range(NBLK // 128):
                for ch in range(4):
                    tp = psBb.tile([128, 128], bf16, tag="trb")
                    nc.tensor.transpose(
                        tp[:], xg1_bf[j][:, ch * 128 : (ch + 1) * 128], identb[:]
                    )
                    for bl in range(4):
                        b = ch * 4 + bl
                        nc.vector.tensor_copy(
                            xgT_k1[:, 8 * j : 8 * (j + 1), b, :],
                            tp[bl * 32 : bl * 32 + 32, :],
                        )

            # reload gathered xg1 into x_sb (waits on AllGather)
            ago_re = ag_out.rearrange("(t p) f -> t p f", p=128)
            for t in range(MT):
                nc.sync.dma_start(x_sb[t][:], ago_re[t])

            # ===== PHASE 4: x_g2^T = 2*(S_blk @ xg1)^T - x^T, direct =====
            for g in range(4):
                ps = psA.tile([128, BC], f32, tag="big")
                for t in range(MT):
                    nc.tensor.matmul(
                        ps[:],
                        x_sb[t][:, g * 128 : (g + 1) * 128],
                        etn[t][:],
                        start=(t == 0),
                        stop=(t == MT - 1),
                    )
                for bl in range(4):
                    b = g * 4 + bl
                    nc.vector.scalar_tensor_tensor(
                        xgT_k2[:, :, b, :],
                        ps[bl * 32 : bl * 32 + 32, :],
                        2.0,
                        xgT_k0[:, :, b, :],
                        ALU.mult,
                        ALU.subtract,
                    )

        # ============ PHASE 5: per-node conv ============
        xgT_v = xgT[:].rearrange("p (h b l) -> p h b l", h=32, b=B)
        for grp in range(NBLK // 8):
            ps = psC.tile([16, 512], f32, tag="cps")
            for s in range(8):
                n = grp * 8 + s
                nh, nl = n // 16, n % 16
                nc.tensor.matmul(
                    ps[0:B, s * DOUT : (s + 1) * DOUT],
                    xgT_v[:, nh, :, nl],
                    wt_v[:, nh, :, nl],
                    start=True,
                    stop=True,
                )
            q = grp % 2
            if q == 0:
                onat = outp.tile([16, 1024], f32, tag="onat")
            if grp % 2 == 0:
                nc.vector.tensor_copy(onat[:, q * 512 : (q + 1) * 512], ps[0:B, :])
            else:
                nc.scalar.activation(
                    onat[:, q * 512 : (q + 1) * 512], ps[0:B, :], AF.Copy
                )
            if q == 1:
                n0 = (grp - 1) * 8
                nc.sync.dma_start(out_d[:, n0 : n0 + 16, :], onat[:])

    nc.compile()
    return nc


def _get_program():
    if "nc" not in _CACHE:
        _CACHE["nc"] = _build_program()
    return _CACHE["nc"]


def _prepare_in_maps(x, node_embeddings, weights_pool, bias_pool, ln_gamma, ln_beta):
    import ml_dtypes

    bf16 = ml_dtypes.bfloat16
    x = np.asarray(x, dtype=np.float32)
    ne = np.asarray(node_embeddings, dtype=np.float32)
    wp = np.asarray(weights_pool, dtype=np.float32).reshape(D, CHEB_K * DIN, DOUT)
    bp = np.asarray(bias_pool, dtype=np.float32)
    gam = np.ascontiguousarray(np.asarray(ln_gamma, dtype=np.float32))
    bet = np.ascontiguousarray(np.asarray(ln_beta, dtype=np.float32))
    ident = np.eye(128, dtype=np.float32)
    identb = np.eye(128, dtype=np.float32).astype(bf16)

    # x transposed to [n, (b c)]
    xt = np.ascontiguousarray(x.transpose(1, 0, 2).reshape(N, BC).astype(bf16))
    # ne rearranged [(p), (t d)]
    ne_re = np.ascontiguousarray(
        ne.reshape(MT, 128, D).transpose(1, 0, 2).reshape(128, MT * D)
    )
    # weights_pool + bias packed: [d, o*(KI+1) + ki], bias at ki=KI
    wpb = np.zeros((D, DOUT * (KI + 1)), dtype=np.float32)
    for o in range(DOUT):
        wpb[:, o * (KI + 1) : o * (KI + 1) + KI] = wp[:, :, o]
        wpb[:, o * (KI + 1) + KI] = bp[:, o]
    wpb = wpb.astype(bf16)

    in_maps = []
    for c in range(NCORES):
        sl = slice(c * NBLK, (c + 1) * NBLK)
        # x^T own block: [c, (n_hi, b, n_lo)]
        xTb = np.ascontiguousarray(
            x[:, sl, :].transpose(2, 1, 0).reshape(DIN, 32, 16, B)
            .transpose(0, 1, 3, 2).reshape(DIN, NBLK * B).astype(bf16)
        )
        neb_re = np.ascontiguousarray(
            ne[sl].reshape(NBLK // 128, 128, D).transpose(1, 0, 2)
            .reshape(128, (NBLK // 128) * D)
        )
        in_maps.append(
            {
                "x_t": xt,
                "xTb": xTb,
                "ne_re": ne_re,
                "neb_re": neb_re,
                "wpb": wpb,
                "gam": gam,
                "bet": bet,
                "ident": ident,
                "identb": identb,
            }
        )
    return in_maps


def kernel(x, node_embeddings, weights_pool, bias_pool, ln_gamma, ln_beta):
    from concourse.bass_utils import run_bass_kernel_spmd

    nc = _get_program()
    in_maps = _prepare_in_maps(
        x, node_embeddings, weights_pool, bias_pool, ln_gamma, ln_beta
    )
    res = run_bass_kernel_spmd(nc, in_maps, list(range(NCORES)))
    out = np.concatenate([res.results[c]["out_blk"] for c in range(NCORES)], axis=1)
    return out
